# revision 38
# baseline (speedup 1.0000x reference)
"""DipoleMomentDecoder on 8 Trainium2 NeuronCores (Bass/Tile).

Strategy
--------
Data-parallel over nodes, but with the nodes globally SORTED by graph id on
the host first.  Each core gets 16384 consecutive sorted nodes, so it only
ever touches a ~256-graph span; per-graph segment sums are then a cheap
one-hot matmul into a per-core 512-graph PSUM window.  The host merges the
8 (overlapping) windows and takes the final norm — O(G) work.

All matmuls run in fp16 (1 cycle/row on the PE, fp32 PSUM accumulate) with a
feature-major layout: activations live as [feature, node] tiles so every
layer's PSUM output is directly the next layer's moving operand — the only
transposes are tiny [5,128] tail blocks done on the PE.

The compiled program is input-independent (per-chunk window offsets are
static under a uniform-distribution assumption with a ±64-graph margin;
the host verifies the margin and falls back to a numpy path if violated).
"""

import os
import sys

import numpy as np

# model dims (hardcoded per spec; anything else falls back to numpy)
N, F, H, G = 131072, 256, 128, 2048
NC = 8
NS = N // NC          # 16384 nodes per core
T = 512               # nodes per column tile
NT = NS // T          # 32 tiles per core
CH = 128              # nodes per segment chunk
NCH = NS // CH        # 128 chunks per core
WIN = 512             # per-core graph window (graphs G0..G0+WIN)
MARGIN = 64           # window head-room below the uniform estimate

_PROG = None          # compiled program cache (input independent)
LAST_RESULTS = None   # BassKernelResults of the most recent device run


def _g0(s):
    return 256 * s - MARGIN


def _b1(c):
    # static block guess for chunk c: nodes ~ 2 graphs/chunk + MARGIN bias
    return min((2 * c + MARGIN) // 128, 2)


def _np_reference(pos, scaler, vector, batch_index, Wg0, W01, b01, W02, b02,
                  Wg1, W11, b11, W12, b12):
    """Pure-numpy fallback (exact reference math)."""
    def gate(s, v, Wg, W1, b1, W2, b2, in_f, out_f):
        proj = np.einsum("nif,fo->nio", v, Wg, optimize=True)
        vec_v, vec_w = proj[..., :in_f], proj[..., in_f:]
        vv = np.sqrt((vec_v ** 2).sum(axis=1))
        x = np.concatenate([s, vv], axis=-1)
        h = x @ W1 + b1
        h = h / (1 + np.exp(-h)) @ W2 + b2
        s_out, g = h[..., :out_f], h[..., out_f:]
        return s_out, g[:, None, :] * vec_w

    q, mu = gate(scaler, vector, Wg0, W01, b01, W02, b02, F, H)
    q = q / (1 + np.exp(-q))
    q, mu = gate(q, mu, Wg1, W11, b11, W12, b12, H, 1)
    mu = mu[..., 0]
    node_mu = mu + q * pos
    gm = np.zeros((G, 3), np.float64)
    np.add.at(gm, np.asarray(batch_index, np.int64), node_mu)
    return np.linalg.norm(gm, axis=-1, keepdims=True).astype(np.float32)


def _import_concourse():
    try:
        import concourse  # noqa: F401
    except ImportError:
        sys.path.insert(0, "/opt/trn_rl_repo")


def _build(nt=NT, sim_compat=False, compile_=True, dbg=False):
    """Trace + compile the (input-independent) 8-core program.

    sim_compat replaces ACT Silu (unimplemented in CoreSim) with
    Sigmoid + DVE multiply; nt<NT builds a shortened program for
    simulator debugging.
    """
    _import_concourse()
    import concourse.tile as tile
    from concourse import bacc, mybir

    f32 = mybir.dt.float32
    f16 = mybir.dt.float16
    AF = mybir.ActivationFunctionType
    ALU = mybir.AluOpType

    nc = bacc.Bacc("TRN2", target_bir_lowering=False, debug=False,
                   num_devices=NC)

    def din(name, shape, dt=f16):
        return nc.dram_tensor(name, shape, dt, kind="ExternalInput").ap()

    # big per-core activations (host-prepared layouts, fp16)
    vt_d = din("vt", [NT, 128, 2, 3, T])          # vector^T tiles
    st_d = din("st", [NT, 128, 2, T])             # scaler^T tiles
    # weights (fp16), partition-first layouts
    wg0_d = din("wg0", [128, 2, 384])
    w01_d = din("w01", [128, 4, 512])
    w02_d = din("w02", [128, 4, 256])
    wg1v_d = din("wg1v", [128, 128])
    wg1w6_d = din("wg1w6", [128, 3, 6])
    w11_d = din("w11", [128, 2, 256])
    w12p_d = din("w12p", [128, 2, 6])
    zeros_d = din("zeros", [128, WIN])
    # fp32 consts / biases / per-node scalars
    b01_d = din("b01", [128, 4], f32)
    b02_d = din("b02", [128, 2], f32)
    b11_d = din("b11", [128, 2], f32)
    b12p_d = din("b12p", [6, 1], f32)
    pos_d = din("pos", [128, NCH * 3], f32)       # node-major pos, chunk cols
    iota_d = din("iota", [128, 256])
    bidx_d = din("bidx", [128, NCH], f32)         # window-relative graph idx
    ident_d = din("ident", [8, 8], f32)

    acc_out = nc.dram_tensor("acc_out", [4, WIN], f32,
                             kind="ExternalOutput").ap()
    dbg_outs = {}
    if dbg:
        for nm_, shp in [("d_nrm0", [128, T]), ("d_nrm1", [128, T]),
                         ("d_vw0", [128, T]), ("d_h10", [128, T]),
                         ("d_q", [128, T]), ("d_gate", [128, T]),
                         ("d_mu0", [128, T]), ("d_nrmd", [128, T]),
                         ("d_h20", [128, T]), ("d_pack", [6, T]),
                         ("d_tps", [128, 6]), ("d_nm", [128, 4]),
                         ("d_L", [128, 256])]:
            dbg_outs[nm_] = nc.dram_tensor(nm_, shp, f32,
                                           kind="ExternalOutput").ap()

    with tile.TileContext(nc) as tc:
        with (
            tc.tile_pool(name="res", bufs=1) as res,
            tc.tile_pool(name="io", bufs=3) as io,
            tc.tile_pool(name="wk", bufs=2) as wk,
            tc.tile_pool(name="ps", bufs=5, space="PSUM") as ps,
            tc.tile_pool(name="pall", bufs=1, space="PSUM") as pallp,
            tc.tile_pool(name="ptp", bufs=1, space="PSUM") as ptp,
            tc.tile_pool(name="pacc", bufs=1, space="PSUM") as pacc,
        ):
            # ---- resident loads -------------------------------------------
            def rtile(ap, shape, dt=f16):
                t = res.tile(shape, dt, tag="r_" + ap.tensor.name)
                nc.sync.dma_start(t[:], ap[:])
                return t

            wg0 = rtile(wg0_d, [128, 2, 384])
            w01 = rtile(w01_d, [128, 4, 512])
            w02 = rtile(w02_d, [128, 4, 256])
            wg1v = rtile(wg1v_d, [128, 128])
            wg1w6 = rtile(wg1w6_d, [128, 3, 6])
            w11 = rtile(w11_d, [128, 2, 256])
            w12p = rtile(w12p_d, [128, 2, 6])
            zeros = rtile(zeros_d, [128, WIN])
            b01 = rtile(b01_d, [128, 4], f32)
            b02 = rtile(b02_d, [128, 2], f32)
            b11 = rtile(b11_d, [128, 2], f32)
            b12p = rtile(b12p_d, [6, 1], f32)
            posr = rtile(pos_d, [128, NCH * 3], f32)
            iota = rtile(iota_d, [128, 256])
            bidx = rtile(bidx_d, [128, NCH], f32)
            ident = rtile(ident_d, [8, 8], f32)

            # ---- per-core graph-sum accumulator ---------------------------
            acc = pacc.tile([4, WIN], f32)
            nc.tensor.matmul(acc[:], zeros[:, 0:4], zeros[:, 0:WIN],
                             start=True, stop=False, skip_group_check=True)

            def norm3(p0, p1, p2, name):
                """sqrt(p0^2+p1^2+p2^2) -> fp16 tile.

                Squares run on ACT (Square is in every table set, so no
                table switch and it doubles as the PSUM->SBUF egress);
                the adds run on the otherwise-idle GpSimd engine.
                """
                cs = []
                for ci, p in enumerate((p0, p1, p2)):
                    c = wk.tile([128, T], f32, tag=f"{name}c{ci}")
                    nc.scalar.activation(c[:], p[:], AF.Square)
                    cs.append(c)
                s0 = wk.tile([128, T], f32, tag=name + "s0")
                nc.gpsimd.tensor_add(s0[:], cs[0][:], cs[1][:])
                nc.gpsimd.tensor_add(s0[:], s0[:], cs[2][:])
                out = wk.tile([128, T], f16, tag=name)
                nc.scalar.activation(out[:], s0[:], AF.Sqrt)
                return out

            def act_silu(p, bias_ap, tag):
                """silu(p + bias) -> fp16; sim_compat avoids ACT Silu."""
                h = wk.tile([128, T], f16, tag=tag)
                if not sim_compat:
                    nc.scalar.activation(h[:], p[:], AF.Silu, bias=bias_ap)
                else:
                    sg = wk.tile([128, T], f32, tag="silu_sg")
                    nc.scalar.activation(sg[:], p[:], AF.Sigmoid,
                                         bias=bias_ap)
                    xb = wk.tile([128, T], f32, tag="silu_xb")
                    nc.vector.tensor_scalar(out=xb[:], in0=p[:],
                                            scalar1=bias_ap, scalar2=None,
                                            op0=ALU.add)
                    nc.vector.tensor_mul(h[:], sg[:], xb[:])
                return h

            def dump(name, ap):
                if dbg and name in dbg_outs:
                    tmp = wk.tile(list(ap.shape), f32, tag="dump" + name)
                    nc.vector.tensor_copy(tmp[:], ap)
                    nc.sync.dma_start(dbg_outs[name][:], tmp[:])

            def tail_phase(cy):
                """Deferred tail of tile cy['t']: D-sqrt, E, F, segment."""
                tt = cy["t"]
                nrmd = wk.tile([128, T], f16, tag="nrmd")
                nc.scalar.activation(nrmd[:], cy["s0d"][:], AF.Sqrt)
                if tt == 0:
                    dump("d_nrmd", nrmd[:])
                # ---- layer E: h2 = silu(x1 @ W11 + b11) -------------------
                x1k = [cy["q"][:], nrmd[:]]
                h2 = []
                for m in range(2):
                    p = ps.tile([128, T], f32, tag="ps")
                    for k in range(2):
                        nc.tensor.matmul(
                            p[:], w11[:, k, 128 * m:128 * (m + 1)], x1k[k],
                            start=(k == 0), stop=(k == 1))
                    h2.append(act_silu(p, b11[:, m:m + 1], f"h2{m}"))
                if tt == 0:
                    dump("d_h20", h2[0][:])
                # ---- layer F: x3 = h2 @ W12 + b12 -------------------------
                pall = cy["pall"]
                for k in range(2):
                    nc.tensor.matmul(pall[:], w12p[:, k, :], h2[k][:],
                                     start=False, stop=(k == 1),
                                     skip_group_check=True)
                # ---- tail: node_mu + segment accumulate -------------------
                # pack rows: 0:3 = w1, 3 = q_final, 4 = gate1, 5 = pad
                pack = wk.tile([6, T], f32, tag="pack")
                nc.vector.tensor_scalar(out=pack[:], in0=pall[:],
                                        scalar1=b12p[:], scalar2=None,
                                        op0=ALU.add)
                if tt == 0:
                    dump("d_pack", pack[:])
                for c4 in range(4):
                    c = 4 * tt + c4
                    tp = ptp.tile([128, 8], f32, tag="tp")
                    nc.tensor.transpose(tp[:, 0:6],
                                        pack[:, 128 * c4:128 * (c4 + 1)],
                                        ident[0:6, 0:6])
                    tps = wk.tile([128, 6], f32, tag="tps")
                    nc.vector.tensor_copy(tps[:], tp[:, 0:6])
                    # node_mu = w1*gate1 + pos*q_final   (node-major)
                    tmp3 = wk.tile([128, 3], f32, tag="tmp3")
                    nc.vector.tensor_scalar(out=tmp3[:], in0=tps[:, 0:3],
                                            scalar1=tps[:, 4:5], scalar2=None,
                                            op0=ALU.mult)
                    nm = wk.tile([128, 4], f16, tag="nm")
                    nc.vector.scalar_tensor_tensor(
                        out=nm[:, 0:3], in0=posr[:, 3 * c:3 * c + 3],
                        scalar=tps[:, 3:4], in1=tmp3[:],
                        op0=ALU.mult, op1=ALU.add)
                    nc.vector.tensor_scalar(out=nm[:, 3:4], in0=tps[:, 3:4],
                                            scalar1=0.0, scalar2=None,
                                            op0=ALU.mult)
                    # one-hot window matmul
                    L = wk.tile([128, 256], f16, tag="L")
                    nc.vector.tensor_scalar(out=L[:], in0=iota[:],
                                            scalar1=bidx[:, c:c + 1],
                                            scalar2=None, op0=ALU.is_equal)
                    if tt == 0 and c4 == 0:
                        dump("d_tps", tps[:])
                        dump("d_nm", nm[:])
                        dump("d_L", L[:])
                    wc = 128 * _b1(c)
                    last = (tt == nt - 1 and c4 == 3)
                    nc.tensor.matmul(acc[:, wc:wc + 256], nm[:], L[:],
                                     start=False, stop=last,
                                     skip_group_check=True)

            carry = None
            for t in range(nt):
                # ---- loads ------------------------------------------------
                vt = io.tile([128, 2, 3, T], f16, tag="vt")
                nc.sync.dma_start(vt[:], vt_d[t])
                st = io.tile([128, 2, T], f16, tag="st")
                nc.sync.dma_start(st[:], st_d[t])

                # ---- layer A: proj = vector @ Wg0 -------------------------
                # m-tiles 0,1 -> vec_v (norm); m-tile 2 -> vec_w
                # squares (ACT, table-neutral) + adds (GpSimd) per m-group
                sums = {}
                vw = []
                for m in (0, 1, 2):
                    pa = []
                    for i in range(3):
                        p = ps.tile([128, T], f32, tag="ps")
                        pa.append(p)
                    for k in range(2):
                        for i in range(3):
                            nc.tensor.matmul(
                                pa[i][:], wg0[:, k, 128 * m:128 * (m + 1)],
                                vt[:, k, i, :],
                                start=(k == 0), stop=(k == 1))
                    if m < 2:
                        sqt = wk.tile([128, 3, T], f32, tag=f"n{m}sq")
                        for ci in range(3):
                            nc.scalar.activation(sqt[:, ci, :], pa[ci][:],
                                                 AF.Square)
                        s0 = wk.tile([128, T], f32, tag=f"n{m}s0")
                        nc.vector.tensor_add(s0[:], sqt[:, 0, :],
                                             sqt[:, 1, :])
                        nc.vector.tensor_add(s0[:], s0[:], sqt[:, 2, :])
                        sums[m] = s0
                    else:
                        for i in range(3):
                            v = wk.tile([128, T], f16, tag=f"vw{i}")
                            nc.vector.tensor_copy(v[:], pa[i][:])
                            vw.append(v)

                # ---- sqrt batch (one table switch) ------------------------
                nrm0 = wk.tile([128, T], f16, tag="nrm0")
                nc.scalar.activation(nrm0[:], sums[0][:], AF.Sqrt)
                nrm1 = wk.tile([128, T], f16, tag="nrm1")
                nc.scalar.activation(nrm1[:], sums[1][:], AF.Sqrt)
                if t == 0:
                    dump("d_nrm0", nrm0[:])
                    dump("d_nrm1", nrm1[:])
                    dump("d_vw0", vw[0][:])
                # deferred tail of the previous tile rides this tile's
                # sqrt/silu table batches
                if carry is not None:
                    tail_phase(carry)

                # ---- layer B: h1 = silu(x @ W01 + b01) --------------------
                xk = [st[:, 0, :], st[:, 1, :], nrm0[:], nrm1[:]]
                h1 = []
                for m in range(4):
                    p = ps.tile([128, T], f32, tag="ps")
                    for k in range(4):
                        nc.tensor.matmul(
                            p[:], w01[:, k, 128 * m:128 * (m + 1)], xk[k],
                            start=(k == 0), stop=(k == 3))
                    h1.append(act_silu(p, b01[:, m:m + 1], f"h1{m}"))

                if t == 0:
                    dump("d_h10", h1[0][:])
                # ---- layer C: x2 = h1 @ W02 + b02 -------------------------
                pc = []
                for m in range(2):
                    p = ps.tile([128, T], f32, tag="ps")
                    for k in range(4):
                        nc.tensor.matmul(
                            p[:], w02[:, k, 128 * m:128 * (m + 1)], h1[k][:],
                            start=(k == 0), stop=(k == 3))
                    pc.append(p)
                q = act_silu(pc[0], b02[:, 0:1], "q")
                gate = wk.tile([128, T], f16, tag="gate")
                nc.vector.tensor_scalar(out=gate[:], in0=pc[1][:],
                                        scalar1=b02[:, 1:2], scalar2=None,
                                        op0=ALU.add)
                mu = []
                for i in range(3):
                    m_ = wk.tile([128, T], f16, tag=f"mu{i}")
                    nc.vector.tensor_mul(m_[:], gate[:], vw[i][:])
                    mu.append(m_)

                if t == 0:
                    dump("d_q", q[:])
                    dump("d_gate", gate[:])
                    dump("d_mu0", mu[0][:])
                # ---- layer D: proj1 = mu @ Wg1 ----------------------------
                # wg1w rows accumulate into pall (finished by layer F in the
                # deferred tail); D's sum-of-squares is also deferred
                pall = pallp.tile([6, T], f32, tag="pall")
                pd = []
                for i in range(3):
                    p = ps.tile([128, T], f32, tag="ps")
                    nc.tensor.matmul(p[:], wg1v[:], mu[i][:],
                                     start=True, stop=True)
                    pd.append(p)
                for i in range(3):
                    nc.tensor.matmul(pall[:], wg1w6[:, i, :], mu[i][:],
                                     start=(i == 0), stop=False,
                                     skip_group_check=True)
                cs = []
                for ci in range(3):
                    c = wk.tile([128, T], f32, tag=f"ndc{ci}")
                    nc.vector.tensor_copy(c[:], pd[ci][:])
                    sq = wk.tile([128, T], f32, tag=f"ndq{ci}")
                    nc.gpsimd.tensor_mul(sq[:], c[:], c[:])
                    cs.append(sq)
                s0d = wk.tile([128, T], f32, tag="nds0")
                nc.gpsimd.tensor_add(s0d[:], cs[0][:], cs[1][:])
                nc.gpsimd.tensor_add(s0d[:], s0d[:], cs[2][:])

                carry = {"t": t, "s0d": s0d, "q": q, "pall": pall}

            tail_phase(carry)

            out_sb = res.tile([4, WIN], f32)
            nc.vector.tensor_copy(out_sb[:], acc[:])
            nc.sync.dma_start(acc_out[:], out_sb[:])

    if compile_:
        nc.compile()
    return nc


def _prep_core(s, perm, g_sorted, pos, scaler, vector):
    """Host-side shard prep: gather into sorted order + device layouts."""
    lo, hi = s * NS, (s + 1) * NS
    idx = perm[lo:hi]
    gs = g_sorted[lo:hi].astype(np.int64)

    c = np.arange(NS) // CH
    b1 = np.minimum((2 * c + MARGIN) // 128, 2)
    # one-hot index within the chunk's 256-wide window; +64 centres the
    # expected position so there is ~±64 graphs of slack on both sides
    j = gs - _g0(s) - 128 * b1 + 64
    if j.min() < 0 or j.max() >= 256:
        return None  # window assumption violated -> caller falls back

    vt = np.ascontiguousarray(
        vector[idx].reshape(NT, T, 3, 2, 128).transpose(0, 4, 3, 2, 1)
    ).astype(np.float16)
    st = np.ascontiguousarray(
        scaler[idx].reshape(NT, T, 2, 128).transpose(0, 3, 2, 1)
    ).astype(np.float16)
    posr = np.ascontiguousarray(
        pos[idx].reshape(NCH, CH, 3).transpose(1, 0, 2).reshape(128, NCH * 3)
    ).astype(np.float32)
    bidx = np.ascontiguousarray(
        j.astype(np.float32).reshape(NCH, CH).T)
    return {"vt": vt, "st": st, "pos": posr, "bidx": bidx}



def _shared_params(Wg0, W01, b01, W02, b02, Wg1, W11, b11, W12, b12):
    """Replicated device params in partition-first fp16 layouts."""
    wg1w6 = np.zeros((128, 3, 6), np.float32)
    for i in range(3):
        wg1w6[:, i, i] = Wg1[:, H]
    w12p = np.zeros((128, 2, 6), np.float32)
    for k in range(2):
        w12p[:, k, 3] = W12[128 * k:128 * (k + 1), 0]
        w12p[:, k, 4] = W12[128 * k:128 * (k + 1), 1]
    b12p = np.zeros((6, 1), np.float32)
    b12p[3, 0], b12p[4, 0] = b12[0], b12[1]
    return {
        "wg0": np.ascontiguousarray(
            Wg0.reshape(2, 128, 384).transpose(1, 0, 2)).astype(np.float16),
        "w01": np.ascontiguousarray(
            W01.reshape(4, 128, 512).transpose(1, 0, 2)).astype(np.float16),
        "w02": np.ascontiguousarray(
            W02.reshape(4, 128, 256).transpose(1, 0, 2)).astype(np.float16),
        "wg1v": np.ascontiguousarray(Wg1[:, :H]).astype(np.float16),
        "wg1w6": wg1w6.astype(np.float16),
        "w11": np.ascontiguousarray(
            W11.reshape(2, 128, 256).transpose(1, 0, 2)).astype(np.float16),
        "w12p": w12p.astype(np.float16),
        "zeros": np.zeros((128, WIN), np.float16),
        "b01": np.ascontiguousarray(b01.reshape(4, 128).T).astype(np.float32),
        "b02": np.ascontiguousarray(b02.reshape(2, 128).T).astype(np.float32),
        "b11": np.ascontiguousarray(b11.reshape(2, 128).T).astype(np.float32),
        "b12p": b12p,
        "iota": np.tile(np.arange(256, dtype=np.float16), (128, 1)),
        "ident": np.eye(8, dtype=np.float32),
    }

def kernel(pos, scaler, vector, batch_index, Wg0, W01, b01, W02, b02,
           Wg1, W11, b11, W12, b12):
    global _PROG, LAST_RESULTS

    args = (pos, scaler, vector, batch_index, Wg0, W01, b01, W02, b02,
            Wg1, W11, b11, W12, b12)
    args = tuple(np.asarray(a) for a in args)
    (pos, scaler, vector, batch_index, Wg0, W01, b01, W02, b02,
     Wg1, W11, b11, W12, b12) = args

    if (pos.shape != (N, 3) or scaler.shape != (N, F)
            or vector.shape != (N, 3, F) or batch_index.shape != (N,)
            or int(batch_index.max(initial=0)) >= G
            or int(batch_index.min(initial=0)) < 0):
        return _np_reference(*[a.astype(np.float64) if a.dtype.kind == "f"
                               else a for a in args])

    bi = batch_index.astype(np.int64)
    perm = np.argsort(bi, kind="stable")
    g_sorted = bi[perm]

    per_core = []
    for s in range(NC):
        d = _prep_core(s, perm, g_sorted, pos.astype(np.float32),
                       scaler.astype(np.float32), vector.astype(np.float32))
        if d is None:
            return _np_reference(*[a.astype(np.float64) if a.dtype.kind == "f"
                                   else a for a in args])
        per_core.append(d)

    shared = _shared_params(Wg0, W01, b01, W02, b02, Wg1, W11, b11, W12, b12)

    _import_concourse()
    from concourse.bass_utils import run_bass_kernel_spmd

    if _PROG is None:
        _PROG = _build()

    in_maps = [dict(shared, **per_core[s]) for s in range(NC)]
    try:
        res = run_bass_kernel_spmd(_PROG, in_maps, list(range(NC)))
    except Exception:
        # transient device/profiling hiccups: retry once without tracing
        os.environ["BASS_NEVER_TRACE"] = "1"
        res = run_bass_kernel_spmd(_PROG, in_maps, list(range(NC)))
        os.environ.pop("BASS_NEVER_TRACE", None)
    LAST_RESULTS = res

    gm = np.zeros((G, 3), np.float64)
    g_all = np.arange(WIN)
    for s in range(NC):
        acc = res.results[s]["acc_out"][0:3]          # [3, WIN]
        g = _g0(s) - 64 + g_all
        m = (g >= 0) & (g < G)
        gm[g[m]] += acc.T[m]
    out = np.linalg.norm(gm, axis=-1, keepdims=True).astype(np.float32)
    return out


# revision 39
# speedup vs baseline: 1.0894x; 1.0894x over previous
"""DipoleMomentDecoder on 8 Trainium2 NeuronCores (Bass/Tile).

Strategy
--------
Data-parallel over nodes, but with the nodes globally SORTED by graph id on
the host first.  Each core gets 16384 consecutive sorted nodes, so it only
ever touches a ~256-graph span; per-graph segment sums are then a cheap
one-hot matmul into a per-core 512-graph PSUM window.  The host merges the
8 (overlapping) windows and takes the final norm — O(G) work.

All matmuls run in fp16 (1 cycle/row on the PE, fp32 PSUM accumulate) with a
feature-major layout: activations live as [feature, node] tiles so every
layer's PSUM output is directly the next layer's moving operand — the only
transposes are tiny [5,128] tail blocks done on the PE.

The compiled program is input-independent (per-chunk window offsets are
static under a uniform-distribution assumption with a ±64-graph margin;
the host verifies the margin and falls back to a numpy path if violated).
"""

import os
import sys

import numpy as np

# model dims (hardcoded per spec; anything else falls back to numpy)
N, F, H, G = 131072, 256, 128, 2048
NC = 8
NS = N // NC          # 16384 nodes per core
T = 512               # nodes per column tile
NT = NS // T          # 32 tiles per core
CH = 128              # nodes per segment chunk
NCH = NS // CH        # 128 chunks per core
WIN = 512             # per-core graph window (graphs G0..G0+WIN)
MARGIN = 64           # window head-room below the uniform estimate

_PROG = None          # compiled program cache (input independent)
LAST_RESULTS = None   # BassKernelResults of the most recent device run


def _g0(s):
    return 256 * s - MARGIN


def _b1(c):
    # static block guess for chunk c: nodes ~ 2 graphs/chunk + MARGIN bias
    return min((2 * c + MARGIN) // 128, 2)


def _np_reference(pos, scaler, vector, batch_index, Wg0, W01, b01, W02, b02,
                  Wg1, W11, b11, W12, b12):
    """Pure-numpy fallback (exact reference math)."""
    def gate(s, v, Wg, W1, b1, W2, b2, in_f, out_f):
        proj = np.einsum("nif,fo->nio", v, Wg, optimize=True)
        vec_v, vec_w = proj[..., :in_f], proj[..., in_f:]
        vv = np.sqrt((vec_v ** 2).sum(axis=1))
        x = np.concatenate([s, vv], axis=-1)
        h = x @ W1 + b1
        h = h / (1 + np.exp(-h)) @ W2 + b2
        s_out, g = h[..., :out_f], h[..., out_f:]
        return s_out, g[:, None, :] * vec_w

    q, mu = gate(scaler, vector, Wg0, W01, b01, W02, b02, F, H)
    q = q / (1 + np.exp(-q))
    q, mu = gate(q, mu, Wg1, W11, b11, W12, b12, H, 1)
    mu = mu[..., 0]
    node_mu = mu + q * pos
    gm = np.zeros((G, 3), np.float64)
    np.add.at(gm, np.asarray(batch_index, np.int64), node_mu)
    return np.linalg.norm(gm, axis=-1, keepdims=True).astype(np.float32)


def _import_concourse():
    try:
        import concourse  # noqa: F401
    except ImportError:
        sys.path.insert(0, "/opt/trn_rl_repo")


def _build(nt=NT, sim_compat=False, compile_=True, dbg=False):
    """Trace + compile the (input-independent) 8-core program.

    sim_compat replaces ACT Silu (unimplemented in CoreSim) with
    Sigmoid + DVE multiply; nt<NT builds a shortened program for
    simulator debugging.
    """
    _import_concourse()
    import concourse.tile as tile
    from concourse import bacc, mybir

    f32 = mybir.dt.float32
    f16 = mybir.dt.float16
    AF = mybir.ActivationFunctionType
    ALU = mybir.AluOpType

    nc = bacc.Bacc("TRN2", target_bir_lowering=False, debug=False,
                   num_devices=NC)

    def din(name, shape, dt=f16):
        return nc.dram_tensor(name, shape, dt, kind="ExternalInput").ap()

    # big per-core activations (host-prepared layouts, fp16)
    vt_d = din("vt", [NT, 128, 2, 3, T])          # vector^T tiles
    st_d = din("st", [NT, 128, 2, T])             # scaler^T tiles
    # weights (fp16), partition-first layouts
    wg0_d = din("wg0", [128, 2, 384])
    w01_d = din("w01", [128, 4, 512])
    w02_d = din("w02", [128, 4, 256])
    wg1v_d = din("wg1v", [128, 128])
    wg1w6_d = din("wg1w6", [128, 3, 6])
    w11_d = din("w11", [128, 2, 256])
    w12p_d = din("w12p", [128, 2, 6])
    zeros_d = din("zeros", [128, WIN])
    # fp32 consts / biases / per-node scalars
    b01_d = din("b01", [128, 4], f32)
    b02_d = din("b02", [128, 2], f32)
    b11_d = din("b11", [128, 2], f32)
    b12p_d = din("b12p", [6, 1], f32)
    pos_d = din("pos", [128, NCH * 3], f32)       # node-major pos, chunk cols
    iota_d = din("iota", [128, 256])
    bidx_d = din("bidx", [128, NCH], f32)         # window-relative graph idx
    ident_d = din("ident", [8, 8], f32)

    acc_out = nc.dram_tensor("acc_out", [4, WIN], f32,
                             kind="ExternalOutput").ap()
    dbg_outs = {}
    if dbg:
        for nm_, shp in [("d_nrm0", [128, T]), ("d_nrm1", [128, T]),
                         ("d_vw0", [128, T]), ("d_h10", [128, T]),
                         ("d_q", [128, T]), ("d_gate", [128, T]),
                         ("d_mu0", [128, T]), ("d_nrmd", [128, T]),
                         ("d_h20", [128, T]), ("d_pack", [6, T]),
                         ("d_tps", [128, 6]), ("d_nm", [128, 4]),
                         ("d_L", [128, 256])]:
            dbg_outs[nm_] = nc.dram_tensor(nm_, shp, f32,
                                           kind="ExternalOutput").ap()

    with tile.TileContext(nc) as tc:
        with (
            tc.tile_pool(name="res", bufs=1) as res,
            tc.tile_pool(name="io", bufs=3) as io,
            tc.tile_pool(name="wk", bufs=2) as wk,
            tc.tile_pool(name="ps", bufs=5, space="PSUM") as ps,
            tc.tile_pool(name="pall", bufs=1, space="PSUM") as pallp,
            tc.tile_pool(name="ptp", bufs=1, space="PSUM") as ptp,
            tc.tile_pool(name="pacc", bufs=1, space="PSUM") as pacc,
        ):
            # ---- resident loads -------------------------------------------
            def rtile(ap, shape, dt=f16):
                t = res.tile(shape, dt, tag="r_" + ap.tensor.name)
                nc.sync.dma_start(t[:], ap[:])
                return t

            wg0 = rtile(wg0_d, [128, 2, 384])
            w01 = rtile(w01_d, [128, 4, 512])
            w02 = rtile(w02_d, [128, 4, 256])
            wg1v = rtile(wg1v_d, [128, 128])
            wg1w6 = rtile(wg1w6_d, [128, 3, 6])
            w11 = rtile(w11_d, [128, 2, 256])
            w12p = rtile(w12p_d, [128, 2, 6])
            zeros = rtile(zeros_d, [128, WIN])
            b01 = rtile(b01_d, [128, 4], f32)
            b02 = rtile(b02_d, [128, 2], f32)
            b11 = rtile(b11_d, [128, 2], f32)
            b12p = rtile(b12p_d, [6, 1], f32)
            posr = rtile(pos_d, [128, NCH * 3], f32)
            iota = rtile(iota_d, [128, 256])
            bidx = rtile(bidx_d, [128, NCH], f32)
            ident = rtile(ident_d, [8, 8], f32)

            # ---- per-core graph-sum accumulator ---------------------------
            acc = pacc.tile([4, WIN], f32)
            nc.tensor.matmul(acc[:], zeros[:, 0:4], zeros[:, 0:WIN],
                             start=True, stop=False, skip_group_check=True)

            def norm3(p0, p1, p2, name):
                """sqrt(p0^2+p1^2+p2^2) -> fp16 tile.

                Squares run on ACT (Square is in every table set, so no
                table switch and it doubles as the PSUM->SBUF egress);
                the adds run on the otherwise-idle GpSimd engine.
                """
                cs = []
                for ci, p in enumerate((p0, p1, p2)):
                    c = wk.tile([128, T], f32, tag=f"{name}c{ci}")
                    nc.scalar.activation(c[:], p[:], AF.Square)
                    cs.append(c)
                s0 = wk.tile([128, T], f32, tag=name + "s0")
                nc.gpsimd.tensor_add(s0[:], cs[0][:], cs[1][:])
                nc.gpsimd.tensor_add(s0[:], s0[:], cs[2][:])
                out = wk.tile([128, T], f16, tag=name)
                nc.scalar.activation(out[:], s0[:], AF.Sqrt)
                return out

            def act_silu(p, bias_ap, tag):
                """silu(p + bias) -> fp16; sim_compat avoids ACT Silu."""
                h = wk.tile([128, T], f16, tag=tag)
                if not sim_compat:
                    nc.scalar.activation(h[:], p[:], AF.Silu, bias=bias_ap)
                else:
                    sg = wk.tile([128, T], f32, tag="silu_sg")
                    nc.scalar.activation(sg[:], p[:], AF.Sigmoid,
                                         bias=bias_ap)
                    xb = wk.tile([128, T], f32, tag="silu_xb")
                    nc.vector.tensor_scalar(out=xb[:], in0=p[:],
                                            scalar1=bias_ap, scalar2=None,
                                            op0=ALU.add)
                    nc.vector.tensor_mul(h[:], sg[:], xb[:])
                return h

            def dump(name, ap):
                if dbg and name in dbg_outs:
                    tmp = wk.tile(list(ap.shape), f32, tag="dump" + name)
                    nc.vector.tensor_copy(tmp[:], ap)
                    nc.sync.dma_start(dbg_outs[name][:], tmp[:])

            def tail_phase(cy):
                """Deferred tail of tile cy['t']: D-sqrt, E, F, segment."""
                tt = cy["t"]
                nrmd = wk.tile([128, T], f16, tag="nrmd")
                nc.scalar.activation(nrmd[:], cy["s0d"][:], AF.Sqrt)
                if tt == 0:
                    dump("d_nrmd", nrmd[:])
                # ---- layer E: h2 = silu(x1 @ W11 + b11) -------------------
                x1k = [cy["q"][:], nrmd[:]]
                h2 = []
                for m in range(2):
                    p = ps.tile([128, T], f32, tag="ps")
                    for k in range(2):
                        nc.tensor.matmul(
                            p[:], w11[:, k, 128 * m:128 * (m + 1)], x1k[k],
                            start=(k == 0), stop=(k == 1))
                    h2.append(act_silu(p, b11[:, m:m + 1], f"h2{m}"))
                if tt == 0:
                    dump("d_h20", h2[0][:])
                # ---- layer F: x3 = h2 @ W12 + b12 -------------------------
                pall = cy["pall"]
                for k in range(2):
                    nc.tensor.matmul(pall[:], w12p[:, k, :], h2[k][:],
                                     start=False, stop=(k == 1),
                                     skip_group_check=True)
                # ---- tail: node_mu + segment accumulate -------------------
                # pack rows: 0:3 = w1, 3 = q_final, 4 = gate1, 5 = pad
                pack = wk.tile([6, T], f32, tag="pack")
                nc.vector.tensor_scalar(out=pack[:], in0=pall[:],
                                        scalar1=b12p[:], scalar2=None,
                                        op0=ALU.add)
                if tt == 0:
                    dump("d_pack", pack[:])
                for c4 in range(4):
                    c = 4 * tt + c4
                    tp = ptp.tile([128, 8], f32, tag="tp")
                    nc.tensor.transpose(tp[:, 0:6],
                                        pack[:, 128 * c4:128 * (c4 + 1)],
                                        ident[0:6, 0:6])
                    tps = wk.tile([128, 6], f32, tag="tps")
                    nc.vector.tensor_copy(tps[:], tp[:, 0:6])
                    # node_mu = w1*gate1 + pos*q_final   (node-major)
                    tmp3 = wk.tile([128, 3], f32, tag="tmp3")
                    nc.vector.tensor_scalar(out=tmp3[:], in0=tps[:, 0:3],
                                            scalar1=tps[:, 4:5], scalar2=None,
                                            op0=ALU.mult)
                    nm = wk.tile([128, 4], f16, tag="nm")
                    nc.vector.scalar_tensor_tensor(
                        out=nm[:, 0:3], in0=posr[:, 3 * c:3 * c + 3],
                        scalar=tps[:, 3:4], in1=tmp3[:],
                        op0=ALU.mult, op1=ALU.add)
                    nc.vector.tensor_scalar(out=nm[:, 3:4], in0=tps[:, 3:4],
                                            scalar1=0.0, scalar2=None,
                                            op0=ALU.mult)
                    # one-hot window matmul
                    L = wk.tile([128, 256], f16, tag="L")
                    nc.vector.tensor_scalar(out=L[:], in0=iota[:],
                                            scalar1=bidx[:, c:c + 1],
                                            scalar2=None, op0=ALU.is_equal)
                    if tt == 0 and c4 == 0:
                        dump("d_tps", tps[:])
                        dump("d_nm", nm[:])
                        dump("d_L", L[:])
                    wc = 128 * _b1(c)
                    last = (tt == nt - 1 and c4 == 3)
                    nc.tensor.matmul(acc[:, wc:wc + 256], nm[:], L[:],
                                     start=False, stop=last,
                                     skip_group_check=True)

            carry = None
            for t in range(nt):
                # ---- loads ------------------------------------------------
                vt = io.tile([128, 2, 3, T], f16, tag="vt")
                nc.sync.dma_start(vt[:], vt_d[t])
                st = io.tile([128, 2, T], f16, tag="st")
                nc.sync.dma_start(st[:], st_d[t])

                # ---- layer A: proj = vector @ Wg0 -------------------------
                # m-tiles 0,1 -> vec_v (norm); m-tile 2 -> vec_w
                # squares (ACT, table-neutral) + adds (GpSimd) per m-group
                sums = {}
                vw = []
                for m in (0, 1, 2):
                    pa = []
                    for i in range(3):
                        p = ps.tile([128, T], f32, tag="ps")
                        pa.append(p)
                    for k in range(2):
                        for i in range(3):
                            nc.tensor.matmul(
                                pa[i][:], wg0[:, k, 128 * m:128 * (m + 1)],
                                vt[:, k, i, :],
                                start=(k == 0), stop=(k == 1))
                    if m < 2:
                        sqt = wk.tile([128, 3, T], f32, tag=f"n{m}sq")
                        for ci in range(3):
                            nc.scalar.activation(sqt[:, ci, :], pa[ci][:],
                                                 AF.Square)
                        s0 = wk.tile([128, T], f32, tag=f"n{m}s0")
                        nc.vector.tensor_reduce(
                            s0[:], sqt[:].rearrange("p a b -> p b a"),
                            axis=mybir.AxisListType.X, op=ALU.add)
                        sums[m] = s0
                    else:
                        for i in range(3):
                            v = wk.tile([128, T], f16, tag=f"vw{i}")
                            nc.vector.tensor_copy(v[:], pa[i][:])
                            vw.append(v)

                # ---- sqrt batch (one table switch) ------------------------
                nrm0 = wk.tile([128, T], f16, tag="nrm0")
                nc.scalar.activation(nrm0[:], sums[0][:], AF.Sqrt)
                nrm1 = wk.tile([128, T], f16, tag="nrm1")
                nc.scalar.activation(nrm1[:], sums[1][:], AF.Sqrt)
                if t == 0:
                    dump("d_nrm0", nrm0[:])
                    dump("d_nrm1", nrm1[:])
                    dump("d_vw0", vw[0][:])
                # deferred tail of the previous tile rides this tile's
                # sqrt/silu table batches
                if carry is not None:
                    tail_phase(carry)

                # ---- layer B: h1 = silu(x @ W01 + b01) --------------------
                xk = [st[:, 0, :], st[:, 1, :], nrm0[:], nrm1[:]]
                h1 = []
                for m in range(4):
                    p = ps.tile([128, T], f32, tag="ps")
                    for k in range(4):
                        nc.tensor.matmul(
                            p[:], w01[:, k, 128 * m:128 * (m + 1)], xk[k],
                            start=(k == 0), stop=(k == 3))
                    h1.append(act_silu(p, b01[:, m:m + 1], f"h1{m}"))

                if t == 0:
                    dump("d_h10", h1[0][:])
                # ---- layer C: x2 = h1 @ W02 + b02 -------------------------
                pc = []
                for m in range(2):
                    p = ps.tile([128, T], f32, tag="ps")
                    for k in range(4):
                        nc.tensor.matmul(
                            p[:], w02[:, k, 128 * m:128 * (m + 1)], h1[k][:],
                            start=(k == 0), stop=(k == 3))
                    pc.append(p)
                q = act_silu(pc[0], b02[:, 0:1], "q")
                gate = wk.tile([128, T], f16, tag="gate")
                nc.vector.tensor_scalar(out=gate[:], in0=pc[1][:],
                                        scalar1=b02[:, 1:2], scalar2=None,
                                        op0=ALU.add)
                mu = []
                for i in range(3):
                    m_ = wk.tile([128, T], f16, tag=f"mu{i}")
                    nc.vector.tensor_mul(m_[:], gate[:], vw[i][:])
                    mu.append(m_)

                if t == 0:
                    dump("d_q", q[:])
                    dump("d_gate", gate[:])
                    dump("d_mu0", mu[0][:])
                # ---- layer D: proj1 = mu @ Wg1 ----------------------------
                # wg1w rows accumulate into pall (finished by layer F in the
                # deferred tail); D's sum-of-squares is also deferred
                pall = pallp.tile([6, T], f32, tag="pall")
                pd = []
                for i in range(3):
                    p = ps.tile([128, T], f32, tag="ps")
                    nc.tensor.matmul(p[:], wg1v[:], mu[i][:],
                                     start=True, stop=True)
                    pd.append(p)
                for i in range(3):
                    nc.tensor.matmul(pall[:], wg1w6[:, i, :], mu[i][:],
                                     start=(i == 0), stop=False,
                                     skip_group_check=True)
                cs = []
                for ci in range(3):
                    c = wk.tile([128, T], f32, tag=f"ndc{ci}")
                    nc.vector.tensor_copy(c[:], pd[ci][:])
                    sq = wk.tile([128, T], f32, tag=f"ndq{ci}")
                    nc.gpsimd.tensor_mul(sq[:], c[:], c[:])
                    cs.append(sq)
                s0d = wk.tile([128, T], f32, tag="nds0")
                nc.gpsimd.tensor_add(s0d[:], cs[0][:], cs[1][:])
                nc.gpsimd.tensor_add(s0d[:], s0d[:], cs[2][:])

                carry = {"t": t, "s0d": s0d, "q": q, "pall": pall}

            tail_phase(carry)

            out_sb = res.tile([4, WIN], f32)
            nc.vector.tensor_copy(out_sb[:], acc[:])
            nc.sync.dma_start(acc_out[:], out_sb[:])

    if compile_:
        nc.compile()
    return nc


def _prep_core(s, perm, g_sorted, pos, scaler, vector):
    """Host-side shard prep: gather into sorted order + device layouts."""
    lo, hi = s * NS, (s + 1) * NS
    idx = perm[lo:hi]
    gs = g_sorted[lo:hi].astype(np.int64)

    c = np.arange(NS) // CH
    b1 = np.minimum((2 * c + MARGIN) // 128, 2)
    # one-hot index within the chunk's 256-wide window; +64 centres the
    # expected position so there is ~±64 graphs of slack on both sides
    j = gs - _g0(s) - 128 * b1 + 64
    if j.min() < 0 or j.max() >= 256:
        return None  # window assumption violated -> caller falls back

    vt = np.ascontiguousarray(
        vector[idx].reshape(NT, T, 3, 2, 128).transpose(0, 4, 3, 2, 1)
    ).astype(np.float16)
    st = np.ascontiguousarray(
        scaler[idx].reshape(NT, T, 2, 128).transpose(0, 3, 2, 1)
    ).astype(np.float16)
    posr = np.ascontiguousarray(
        pos[idx].reshape(NCH, CH, 3).transpose(1, 0, 2).reshape(128, NCH * 3)
    ).astype(np.float32)
    bidx = np.ascontiguousarray(
        j.astype(np.float32).reshape(NCH, CH).T)
    return {"vt": vt, "st": st, "pos": posr, "bidx": bidx}



def _shared_params(Wg0, W01, b01, W02, b02, Wg1, W11, b11, W12, b12):
    """Replicated device params in partition-first fp16 layouts."""
    wg1w6 = np.zeros((128, 3, 6), np.float32)
    for i in range(3):
        wg1w6[:, i, i] = Wg1[:, H]
    w12p = np.zeros((128, 2, 6), np.float32)
    for k in range(2):
        w12p[:, k, 3] = W12[128 * k:128 * (k + 1), 0]
        w12p[:, k, 4] = W12[128 * k:128 * (k + 1), 1]
    b12p = np.zeros((6, 1), np.float32)
    b12p[3, 0], b12p[4, 0] = b12[0], b12[1]
    return {
        "wg0": np.ascontiguousarray(
            Wg0.reshape(2, 128, 384).transpose(1, 0, 2)).astype(np.float16),
        "w01": np.ascontiguousarray(
            W01.reshape(4, 128, 512).transpose(1, 0, 2)).astype(np.float16),
        "w02": np.ascontiguousarray(
            W02.reshape(4, 128, 256).transpose(1, 0, 2)).astype(np.float16),
        "wg1v": np.ascontiguousarray(Wg1[:, :H]).astype(np.float16),
        "wg1w6": wg1w6.astype(np.float16),
        "w11": np.ascontiguousarray(
            W11.reshape(2, 128, 256).transpose(1, 0, 2)).astype(np.float16),
        "w12p": w12p.astype(np.float16),
        "zeros": np.zeros((128, WIN), np.float16),
        "b01": np.ascontiguousarray(b01.reshape(4, 128).T).astype(np.float32),
        "b02": np.ascontiguousarray(b02.reshape(2, 128).T).astype(np.float32),
        "b11": np.ascontiguousarray(b11.reshape(2, 128).T).astype(np.float32),
        "b12p": b12p,
        "iota": np.tile(np.arange(256, dtype=np.float16), (128, 1)),
        "ident": np.eye(8, dtype=np.float32),
    }

def kernel(pos, scaler, vector, batch_index, Wg0, W01, b01, W02, b02,
           Wg1, W11, b11, W12, b12):
    global _PROG, LAST_RESULTS

    args = (pos, scaler, vector, batch_index, Wg0, W01, b01, W02, b02,
            Wg1, W11, b11, W12, b12)
    args = tuple(np.asarray(a) for a in args)
    (pos, scaler, vector, batch_index, Wg0, W01, b01, W02, b02,
     Wg1, W11, b11, W12, b12) = args

    if (pos.shape != (N, 3) or scaler.shape != (N, F)
            or vector.shape != (N, 3, F) or batch_index.shape != (N,)
            or int(batch_index.max(initial=0)) >= G
            or int(batch_index.min(initial=0)) < 0):
        return _np_reference(*[a.astype(np.float64) if a.dtype.kind == "f"
                               else a for a in args])

    bi = batch_index.astype(np.int64)
    perm = np.argsort(bi, kind="stable")
    g_sorted = bi[perm]

    per_core = []
    for s in range(NC):
        d = _prep_core(s, perm, g_sorted, pos.astype(np.float32),
                       scaler.astype(np.float32), vector.astype(np.float32))
        if d is None:
            return _np_reference(*[a.astype(np.float64) if a.dtype.kind == "f"
                                   else a for a in args])
        per_core.append(d)

    shared = _shared_params(Wg0, W01, b01, W02, b02, Wg1, W11, b11, W12, b12)

    _import_concourse()
    from concourse.bass_utils import run_bass_kernel_spmd

    if _PROG is None:
        _PROG = _build()

    in_maps = [dict(shared, **per_core[s]) for s in range(NC)]
    try:
        res = run_bass_kernel_spmd(_PROG, in_maps, list(range(NC)))
    except Exception:
        # transient device/profiling hiccups: retry once without tracing
        os.environ["BASS_NEVER_TRACE"] = "1"
        res = run_bass_kernel_spmd(_PROG, in_maps, list(range(NC)))
        os.environ.pop("BASS_NEVER_TRACE", None)
    LAST_RESULTS = res

    gm = np.zeros((G, 3), np.float64)
    g_all = np.arange(WIN)
    for s in range(NC):
        acc = res.results[s]["acc_out"][0:3]          # [3, WIN]
        g = _g0(s) - 64 + g_all
        m = (g >= 0) & (g < G)
        gm[g[m]] += acc.T[m]
    out = np.linalg.norm(gm, axis=-1, keepdims=True).astype(np.float32)
    return out
